# revision 35
# baseline (speedup 1.0000x reference)
"""BottleneckLSTMCell fused kernel for 8 Trainium2 NeuronCores.

Sharding: data-parallel over batch (B=8 -> 1 image per core). Each core runs
the full cell for its image. 273.8us HW (v2.12), from 364.7us baseline.

v2.12 design (every step trace-driven; see work/NOTES.md):
- Lag-2 pipeline: round r runs Wy for slab ra=r, dwi/gates/LSTM for slab
  rb=r-2, and the off-PE x-conv for slab rx=r+3 (slabs 0-2 precomputed as
  PE diag matmuls in the prologue while weights stream). The lag means the
  dwi diag MMs read i_pad rows written a full round earlier - PE never
  waits on same-round ACT writes.
- PE: Wy (28 MMs) + dwi diag (36; 27 from round 6 on) + gates (64, into
  2-bank PSUM pair tiles) + dwx chunk-0 diag (9, rounds 0-4) per round at
  ~216ns/MM, N=512. ~22 dummy warm-up MMs at t=0 hold the HAM activity
  window so real MMs start at K=8/8 (2.4 GHz); it stays warm to the end.
- ACT: i_pad writes (Identity+bias from PSUM), b copies, sigmoids batched
  per bank-pair ((i,f) as one FD=1024 ACTIVATE), relu for the c gate.
  Keeping ACT *under* ~16us/round matters more than anything: it is the
  PSUM-recycle path that paces the PE.
- DVE: dwx chunk 1/2 product+add chains (rounds 0-4); from round 6 the
  chains are done and dwi chunk 0 moves here (9 tensor_mul against
  broadcast tap weights + add chain) - its weights are DMA'd OVER dwxb
  slots 0-8 once the last chain has read them. LSTM pointwise all bf16
  (u1/u2/cc/clip via 2-op tensor_scalar/ch).
- Outputs cc/ch are written bf16 (halves store DMA); host casts to f32.
  ch stores ride the ACT DMA queue, everything else the sync queue.
- x loads are single contiguous 1320B-per-partition lines (a slab's 10
  rows are contiguous in the flat 66x66 image); the strided [10,66] form
  generated 1280 tiny descriptors and arrived ~3x late, which stalled the
  ACT queue head and cascaded into PE gaps.
"""

import sys

if '/opt/trn_rl_repo' not in sys.path:
    sys.path.insert(0, '/opt/trn_rl_repo')

import numpy as np
import ml_dtypes

import concourse.bass as bass  # noqa: F401
from concourse import bacc
import concourse.mybir as mybir
from concourse.tile import TileContext
from concourse.bass_utils import run_bass_kernel_spmd

F32 = mybir.dt.float32
BF = mybir.dt.bfloat16
AF = mybir.ActivationFunctionType
ALU = mybir.AluOpType
BF_NP = ml_dtypes.bfloat16

B, CIN, CH, HW = 8, 320, 512, 64
PIX = HW * HW          # 4096
NCORES = 8
NCHUNK = 8             # spatial slabs of 8 rows (512 px)
XCH = [128, 128, 64]   # x channel chunk sizes (320)

TAPS = [(t // 3 - 1, t % 3 - 1) for t in range(9)]
FLAT = "p a b -> p (a b)"


def build_nc():
    nc = bacc.Bacc(None, target_bir_lowering=False, num_devices=NCORES)

    xd = nc.dram_tensor("x", (CIN, 66 * 66), BF, kind="ExternalInput")
    hd = nc.dram_tensor("h", (128, 4, PIX), BF, kind="ExternalInput")
    cd = nc.dram_tensor("c", (128, 4, PIX), BF, kind="ExternalInput")
    wyd = nc.dram_tensor("wy", (128, 7, 512), BF, kind="ExternalInput")
    wybd = nc.dram_tensor("wyb", (128, 4), F32, kind="ExternalInput")
    wgd = nc.dram_tensor("wg", (128, 16, 512), BF, kind="ExternalInput")
    dwidd = nc.dram_tensor("dwid", (128, 4, 1152), BF, kind="ExternalInput")
    dwxdgd = nc.dram_tensor("dwxdg", (128, 3, 1152), BF, kind="ExternalInput")
    dwxbd = nc.dram_tensor("dwxb", (128, 18, 528), BF, kind="ExternalInput")
    dwibd = nc.dram_tensor("dwib", (128, 9, 528), BF, kind="ExternalInput")
    ccd = nc.dram_tensor("occ", (CH, PIX), BF, kind="ExternalOutput")
    chd = nc.dram_tensor("och", (CH, PIX), BF, kind="ExternalOutput")

    x_ap, h_ap, c_ap = xd.ap(), hd.ap(), cd.ap()
    cc_ap, ch_ap = ccd.ap(), chd.ap()

    with TileContext(nc) as tc:
        with tc.tile_pool(name="persist", bufs=1) as pp, \
             tc.tile_pool(name="sA", bufs=2) as sA, \
             tc.tile_pool(name="sB", bufs=2) as sB, \
             tc.tile_pool(name="psi", bufs=2, space="PSUM") as psi_p, \
             tc.tile_pool(name="psg", bufs=2, space="PSUM") as psg_p, \
             tc.tile_pool(name="psb", bufs=2, space="PSUM") as psb_p:

            # ------------- persistent tiles -------------
            i_pad = [pp.tile([128, 66, 66], BF, tag=f"ipad{m}", name=f"ipad{m}")
                     for m in range(4)]
            wy_t = pp.tile([128, 7, 512], BF, tag="wy", name="wy")
            wyb_t = pp.tile([128, 4], F32, tag="wyb", name="wyb")
            wg_t = pp.tile([128, 16, 512], BF, tag="wg", name="wg")
            dwid_t = pp.tile([128, 4, 1152], BF, tag="dwid", name="dwid")
            dwxdg_t = pp.tile([128, 3, 1152], BF, tag="dwxdg", name="dwxdg")
            dwxb_t = pp.tile([128, 18, 528], BF, tag="dwxb", name="dwxb")
            dwib_t = pp.tile([128, 9, 528], BF, tag="dwib", name="dwib")
            scr_t = pp.tile([128, 4], BF, tag="scr", name="scr")
            junk_t = pp.tile([128, 512], BF, tag="junk", name="junk")

            # ------------- helpers -------------
            def dma_h(n):
                t = sB.tile([128, 4, 512], BF, tag="h", name="h")
                nc.sync.dma_start(out=t[:], in_=h_ap[:, :, 512 * n:512 * (n + 1)])
                return t

            def dma_c(n):
                t = sB.tile([128, 4, 512], BF, tag="c", name="c")
                nc.sync.dma_start(out=t[:], in_=c_ap[:, :, 512 * n:512 * (n + 1)])
                return t

            def dma_xpad(n, eng=None):
                # a slab's 10 rows are contiguous in DRAM (flat 66*66 image),
                # so each load is one 1320B line per partition instead of
                # 10 tiny 132B descriptors
                eng = eng or nc.sync
                ts = []
                for ci in range(3):
                    pc = XCH[ci]
                    xp = sA.tile([128, 10, 66], BF, tag=f"xp{ci}",
                                 name=f"xp{ci}", bufs=2)
                    eng.dma_start(
                        out=xp[:pc, :, :].rearrange(FLAT),
                        in_=x_ap[128 * ci:128 * ci + pc,
                                 528 * n:528 * n + 660])
                    ts.append(xp)
                return ts

            def xwin(xp, pc, t):
                dy, dx = TAPS[t]
                return xp[:pc, 1 + dy:9 + dy, 1 + dx:65 + dx]

            def iwin(ci, n, t):
                dy, dx = TAPS[t]
                r0 = 8 * n
                return i_pad[ci][:, 1 + r0 + dy:9 + r0 + dy, 1 + dx:65 + dx]

            def tt_chain(tag, xw_tile, xp, ci, pc=128):
                """dwx chunk on DVE: 9 bf16 tensor_mul products against the
                pre-broadcast weight tiles, then an 8-add chain. All
                contiguous flat ops (elements 1..526)."""
                prod = sA.tile([128, 9, 528], BF, tag="pd",
                               name="pd", bufs=1)
                xf = xp[:pc, :, :].rearrange(FLAT)
                of = xw_tile[:pc, :, :].rearrange(FLAT)
                for t in range(9):
                    dy, dx = TAPS[t]
                    s = 66 * (1 + dy) + dx
                    nc.vector.tensor_mul(
                        prod[:pc, t, 0:526], xf[:, s + 1:s + 527],
                        dwxb_t[:pc, 9 * (ci - 1) + t, 0:526])
                tmps = [sA.tile([128, 528], BF, tag=f"tc{tag}{j}",
                                name=f"tc{tag}{j}") for j in range(2)]
                nc.vector.tensor_add(tmps[0][:pc, 0:526], prod[:pc, 0, 0:526],
                                     prod[:pc, 1, 0:526])
                for j in range(2, 8):
                    nc.vector.tensor_add(tmps[(j + 1) % 2][:pc, 0:526],
                                         tmps[j % 2][:pc, 0:526],
                                         prod[:pc, j, 0:526])
                nc.vector.tensor_add(of[:, 0:526], tmps[0][:pc, 0:526],
                                     prod[:pc, 8, 0:526])

            def new_xw():
                # 4 generations alive: slab ra (reading), ra+1/ra+2 (ready),
                # rx=ra+3 (being written)
                return [sA.tile([128, 8, 66], BF, tag=f"xw{ci}",
                                name=f"xw{ci}", bufs=3) for ci in range(3)]

            def x0_emit(n, xp0, xw0):
                """dwx chunk 0 of slab n as PE diag matmuls + one ACT copy
                (9 MMs ~= 1.9us PE; the 9-ACT-product + 8-Pool-add version
                overloaded ACT in exactly the pipeline-fill rounds)."""
                psb = psb_p.tile([128, 8, 64], F32, tag="psb", name="psb")
                for t in range(9):
                    nc.tensor.matmul(
                        psb[:, :, :], dwxdg_t[:, 0, 128 * t:128 * (t + 1)],
                        xwin(xp0, 128, t), start=(t == 0), stop=(t == 8))
                nc.scalar.copy(xw0[:, :, 0:64], psb[:, :, :])

            def emit_wy(n, h_sb, xw_sb):
                """i = Wy @ [h; xw] + bias -> i_pad interior rows (bf16)."""
                r0 = 8 * n
                for m in range(4):
                    ps = psi_p.tile([128, 512], F32, tag="psi", name="psi")
                    for k in range(4):
                        nc.tensor.matmul(
                            ps[:, :], wy_t[:, k, 128 * m:128 * (m + 1)],
                            h_sb[:, k, :], start=(k == 0), stop=False)
                    for j in range(3):
                        pc = XCH[j]
                        nc.tensor.matmul(
                            ps[:, :], wy_t[:pc, 4 + j, 128 * m:128 * (m + 1)],
                            xw_sb[j][:pc, :, 0:64], start=False, stop=(j == 2))
                    nc.scalar.activation(
                        i_pad[m][:, 1 + r0:9 + r0, 1:65], ps[:, :],
                        AF.Identity, bias=wyb_t[:, m:m + 1], scale=1.0)

            # ---------------- prologue ----------------
            # PE warm-up: ~20 dummy matmuls on a junk tile keep the PE HAM
            # activity window busy while the first DMAs stream, so the real
            # matmuls start at K=8/8 (2.4 GHz) instead of cold.
            nc.vector.memset(junk_t[:, :], 0.0)
            for _ in range(22):
                psw = psb_p.tile([128, 8, 64], F32, tag="psb", name="psb")
                nc.tensor.matmul(psw[:, :, :], junk_t[:, 0:128],
                                 junk_t[:, :].rearrange("p (a b) -> p a b",
                                                        a=8, b=64),
                                 start=True, stop=True)

            # DMA order: first the tensors gating the first diag MMs, then
            # the Wy weights, then everything else. x loads go through the
            # ACT DMA queue in parallel with the sync-queue weights.
            # interleave diag-weight and x chunks so the first diag MM's two
            # inputs are the first two transfers on the queue
            xp0 = []
            for ci in range(3):
                nc.sync.dma_start(out=dwxdg_t[:, ci, :],
                                  in_=dwxdgd.ap()[:, ci, :])
                pc = XCH[ci]
                xp = sA.tile([128, 10, 66], BF, tag=f"xp{ci}",
                             name=f"xp{ci}", bufs=2)
                nc.sync.dma_start(out=xp[:pc, :, :].rearrange(FLAT),
                                  in_=x_ap[128 * ci:128 * ci + pc, 0:660])
                xp0.append(xp)
            xp1 = dma_xpad(1)
            nc.sync.dma_start(out=wy_t[:], in_=wyd.ap())
            h0 = dma_h(0)
            nc.sync.dma_start(out=wyb_t[:], in_=wybd.ap())
            xp2 = dma_xpad(2)
            nc.sync.dma_start(out=dwid_t[:], in_=dwidd.ap())
            nc.sync.dma_start(out=wg_t[:], in_=wgd.ap())
            nc.sync.dma_start(out=dwxb_t[:], in_=dwxbd.ap())
            nc.sync.dma_start(out=dwib_t[:], in_=dwibd.ap())
            xp3 = dma_xpad(3)

            # warm the ACT function tables while DMAs stream
            nc.vector.memset(scr_t[:, :], 0.0)
            nc.scalar.activation(scr_t[:, 0:1], scr_t[:, 1:2], AF.Sigmoid)
            nc.scalar.activation(scr_t[:, 2:3], scr_t[:, 3:4], AF.Relu)

            # zero i_pad halo borders (rows 0/65, cols 0/65)
            for m in range(4):
                eng = nc.vector if m < 2 else nc.gpsimd
                eng.memset(i_pad[m][:, 0, :], 0.0)
                eng.memset(i_pad[m][:, 65, :], 0.0)
                eng.memset(i_pad[m][:, 1:65, 0], 0.0)
                eng.memset(i_pad[m][:, 1:65, 65], 0.0)

            # prologue x-conv for slabs 0-2 entirely as PE diag matmuls
            # (PE is otherwise idle while Wy/gate weights stream in); ACT
            # copies PSUM -> xw bf16
            def dwx_diag(xps):
                xw = new_xw()
                for ci in range(3):
                    pc = XCH[ci]
                    psb = psb_p.tile([128, 8, 64], F32, tag="psb", name="psb")
                    for t in range(9):
                        nc.tensor.matmul(
                            psb[:pc, :, :],
                            dwxdg_t[:pc, ci, pc * t:pc * (t + 1)],
                            xwin(xps[ci], pc, t), start=(t == 0), stop=(t == 8))
                    nc.scalar.copy(xw[ci][:pc, :, 0:64], psb[:pc, :, :])
                return xw

            xw_sb = {0: dwx_diag(xp0), 1: dwx_diag(xp1), 2: dwx_diag(xp2)}
            h_sb = {0: h0}
            xp_sb = {3: xp3}

            # ---------------- rounds ----------------
            # ra = Wy stage slab, rb = dwi/gate/LSTM stage slab (lag 2),
            # rx = dwx stage slab (3 ahead; slabs 0-2 precomputed)
            for r in range(NCHUNK + 2):
                ra = r
                rb = r - 2
                rx = r + 3

                # DMAs for this round
                if rb >= 0:
                    c_t = dma_c(rb)
                if ra + 1 < NCHUNK:
                    h_sb[ra + 1] = dma_h(ra + 1)
                if rx + 1 < NCHUNK:
                    xp_sb[rx + 1] = dma_xpad(rx + 1)

                # PE: Wy matmuls + i_pad for slab ra.
                if ra < NCHUNK:
                    emit_wy(ra, h_sb.pop(ra), xw_sb.pop(ra))

                # From round 6 on the DVE chains are finished, so dwi chunk 0
                # moves to the idle DVE (9 tensor_mul products + add chain),
                # shaving 9 diag MMs off the PE. The i-tap broadcast weights
                # are DMA'd over dwxb slots 0-8 (free after the last chain)
                # at round 5.
                offload_b0 = rb >= 2
                if rb >= 0 and offload_b0:
                    pr = sA.tile([128, 9, 528], BF, tag="pd", name="pd",
                                 bufs=1)
                    r0 = 8 * rb
                    ipf = i_pad[0][:, r0:r0 + 10, :].rearrange(FLAT)
                    for t in range(9):
                        dy, dx = TAPS[t]
                        s = 66 * (1 + dy) + dx
                        nc.vector.tensor_mul(
                            pr[:, t, 0:526], ipf[:, s + 1:s + 527],
                            dwib_t[:, t, 0:526])
                    b0 = sB.tile([128, 8, 66], BF, tag="b0", name="b0")
                    btm = [sA.tile([128, 528], BF, tag=f"b0t{j}",
                                   name=f"b0t{j}", bufs=1) for j in range(2)]
                    nc.vector.tensor_add(btm[0][:, 0:526], pr[:, 0, 0:526],
                                         pr[:, 1, 0:526])
                    for j in range(2, 8):
                        nc.vector.tensor_add(btm[(j + 1) % 2][:, 0:526],
                                             btm[j % 2][:, 0:526],
                                             pr[:, j, 0:526])
                    nc.vector.tensor_add(
                        b0[:, :, :].rearrange(FLAT)[:, 0:526],
                        btm[0][:, 0:526], pr[:, 8, 0:526])

                # DVE: dwx chunks 1/2 of slab rx
                if rx < NCHUNK:
                    xw_next = new_xw()
                    tt_chain("xc1", xw_next[1], xp_sb[rx][1], 1)
                    tt_chain("xc2", xw_next[2], xp_sb[rx][2], 2, pc=64)

                # PE: dwi chunks of slab rb as diag matmuls; i_pad rows for
                # rb+1 (the bottom halo) were written LAST round, so no
                # same-round ACT dependency.
                if rb >= 0:
                    bt = {}
                    for ci in range(1 if offload_b0 else 0, 4):
                        psb = psb_p.tile([128, 8, 64], F32, tag="psb",
                                         name="psb")
                        for t in range(9):
                            nc.tensor.matmul(
                                psb[:, :, :],
                                dwid_t[:, ci, 128 * t:128 * (t + 1)],
                                iwin(ci, rb, t), start=(t == 0), stop=(t == 8))
                        b = sB.tile([128, 8, 64], BF, tag=f"b{ci}",
                                    name=f"b{ci}")
                        nc.scalar.copy(b[:, :, :], psb[:, :, :])
                        bt[ci] = b[:, :, :]
                    if offload_b0:
                        bt[0] = b0[:, :, 0:64]

                    # PE gates (paired PSUM banks) + ACT batched sigmoids +
                    # DVE LSTM pointwise per m-chunk. When b0 is off-PE the
                    # contraction order ends with chunk 0 for maximum slack.
                    KORD = (1, 2, 3, 0) if offload_b0 else (0, 1, 2, 3)
                    for m in range(4):
                        # pair A: gates (i, f) -> one batched sigmoid
                        pgA = psg_p.tile([128, 2, 512], F32, tag="psg",
                                         name="psg")
                        for g in range(2):
                            for j, k in enumerate(KORD):
                                nc.tensor.matmul(
                                    pgA[:, g, :],
                                    wg_t[:, 4 * g + k, 128 * m:128 * (m + 1)],
                                    bt[k],
                                    start=(j == 0), stop=(j == 3))
                        st_if = sB.tile([128, 2, 512], BF, tag="sif",
                                        name="sif", bufs=3)
                        nc.scalar.activation(st_if[:, :, :], pgA[:, :, :],
                                             AF.Sigmoid)
                        # pair B: gates (o, c) -> sigmoid + relu
                        pgB = psg_p.tile([128, 2, 512], F32, tag="psg",
                                         name="psg")
                        for g in range(2, 4):
                            for j, k in enumerate(KORD):
                                nc.tensor.matmul(
                                    pgB[:, g - 2, :],
                                    wg_t[:, 4 * g + k, 128 * m:128 * (m + 1)],
                                    bt[k],
                                    start=(j == 0), stop=(j == 3))
                        st_o = sB.tile([128, 512], BF, tag="so",
                                       name="so", bufs=3)
                        nc.scalar.activation(st_o[:, :], pgB[:, 0, :],
                                             AF.Sigmoid)
                        st_cr = sB.tile([128, 512], BF, tag="scr2",
                                        name="scr2", bufs=3)
                        nc.scalar.activation(st_cr[:, :], pgB[:, 1, :],
                                             AF.Relu)

                        # DVE LSTM: all bf16; none of it holds PSUM banks
                        u1 = sB.tile([128, 512], BF, tag="u1", name="u1",
                                     bufs=3)
                        nc.vector.tensor_mul(u1[:, :], st_if[:, 1, :],
                                             c_t[:, m, :])
                        u2 = sB.tile([128, 512], BF, tag="u2", name="u2",
                                     bufs=3)
                        nc.vector.scalar_tensor_tensor(
                            out=u2[:, :], in0=st_cr[:, :], scalar=6.0,
                            in1=st_if[:, 0, :], op0=ALU.min, op1=ALU.mult)
                        cc_t = sB.tile([128, 512], BF, tag="cc",
                                       name="cc", bufs=3)
                        nc.vector.tensor_add(cc_t[:, :], u1[:, :], u2[:, :])
                        nc.sync.dma_start(
                            out=cc_ap[128 * m:128 * (m + 1),
                                      512 * rb:512 * (rb + 1)],
                            in_=cc_t[:])
                        rcc = sB.tile([128, 512], BF, tag="rcc",
                                      name="rcc", bufs=3)
                        nc.vector.tensor_scalar(
                            out=rcc[:, :], in0=cc_t[:, :], scalar1=6.0,
                            scalar2=0.0, op0=ALU.min, op1=ALU.max)
                        ch_t = sB.tile([128, 512], BF, tag="ch",
                                       name="ch", bufs=3)
                        nc.vector.tensor_mul(ch_t[:, :], rcc[:, :],
                                             st_o[:, :])
                        nc.scalar.dma_start(
                            out=ch_ap[128 * m:128 * (m + 1),
                                      512 * rb:512 * (rb + 1)],
                            in_=ch_t[:])

                # dwx chunk 0 of slab rx: ACT products + Pool add chain,
                # deprioritized so the scheduler parks them behind the
                # latency-critical ACT/Pool work (3 rounds of slack).
                if rx < NCHUNK:
                    with tc.high_priority(offset=-4000):
                        x0_emit(rx, xp_sb[rx][0], xw_next[0])
                    xw_sb[rx] = xw_next
                    del xp_sb[rx]

    nc.compile()
    return nc


def pack_weights(W_dw, W_dwb, Wy, Wy_b, Wi, Wbi, Wbf, Wbc, Wbo):
    WyT = Wy[:, :, 0, 0].T.astype(np.float32)  # (832, 512) lhsT
    wy = np.zeros((128, 7, 512), np.float32)
    for k in range(4):  # h chunks first
        wy[:, k, :] = WyT[320 + 128 * k:320 + 128 * (k + 1), :]
    for k in range(2):
        wy[:, 4 + k, :] = WyT[128 * k:128 * (k + 1), :]
    wy[:64, 6, :] = WyT[256:320, :]

    wyb = (Wy_b + Wy[:, :320, 0, 0] @ W_dwb).astype(np.float32)
    wyb = np.ascontiguousarray(wyb.reshape(4, 128).T)

    # gate order (i, f, o, c) so (i,f) and (o,c) share PSUM bank pairs
    wg = np.zeros((128, 16, 512), np.float32)
    for g, W in enumerate([Wbi, Wbf, Wbo, Wbc]):
        lhsT = W[:, :, 0, 0].T.astype(np.float32)  # (512 in, 512 out)
        for k in range(4):
            wg[:, 4 * g + k, :] = lhsT[128 * k:128 * (k + 1), :]

    wtap_x = W_dw[:, 0].reshape(CIN, 9)  # (c, t) tap-major (dy,dx)

    # pre-broadcast bf16 tap weights for the DVE tensor_mul products:
    # slots 0-17 = x chunks 1/2, slots 18-26 = i chunk 0
    wtap_i = Wi[:, 0].reshape(CH, 9)
    dwxb = np.zeros((128, 18, 528), np.float32)
    for ci in (1, 2):
        pc = XCH[ci]
        for t in range(9):
            dwxb[:pc, 9 * (ci - 1) + t, :] = \
                wtap_x[128 * ci:128 * ci + pc, t][:, None]

    dwib = np.zeros((128, 9, 528), np.float32)
    for t in range(9):
        dwib[:, t, :] = wtap_i[:128, t][:, None]

    dwid = np.zeros((128, 4, 1152), np.float32)
    idx = np.arange(128)
    for ci in range(4):
        for t in range(9):
            dwid[idx, ci, 128 * t + idx] = wtap_i[128 * ci + idx, t]
    dwxdg = np.zeros((128, 3, 1152), np.float32)
    for ci in range(3):
        pc = XCH[ci]
        ic = np.arange(pc)
        for t in range(9):
            dwxdg[ic, ci, pc * t + ic] = wtap_x[128 * ci + ic, t]

    return {
        "wy": wy.astype(BF_NP), "wyb": wyb, "wg": wg.astype(BF_NP),
        "dwid": dwid.astype(BF_NP),
        "dwxdg": dwxdg.astype(BF_NP), "dwxb": dwxb.astype(BF_NP),
        "dwib": dwib.astype(BF_NP),
    }


_CACHE = {}


def _get_nc():
    if "nc" not in _CACHE:
        _CACHE["nc"] = build_nc()
    return _CACHE["nc"]


def run(inputs, trace=False, tmpdir=None):
    """inputs: dict as from setup_inputs(). Returns ((ch, cc), results_obj)."""
    inp = {k: np.asarray(v, np.float32) for k, v in inputs.items()}
    packed = pack_weights(
        inp["W_dw"], inp["W_dwb"], inp["Wy"], inp["Wy_b"], inp["Wi"],
        inp["Wbi"], inp["Wbf"], inp["Wbc"], inp["Wbo"],
    )
    xpad_host = np.zeros((B, CIN, 66, 66), np.float32)
    xpad_host[:, :, 1:65, 1:65] = inp["x"]
    xpad_host = xpad_host.astype(BF_NP).reshape(B, CIN, 66 * 66)
    # h/c device layout: [c_low(128), k_chunk(4), pix]
    h_host = np.ascontiguousarray(
        inp["h"].reshape(B, 4, 128, PIX).transpose(0, 2, 1, 3)).astype(BF_NP)
    c_host = np.ascontiguousarray(
        inp["c"].reshape(B, 4, 128, PIX).transpose(0, 2, 1, 3)).astype(BF_NP)
    in_maps = []
    for b in range(B):
        in_maps.append({
            "x": xpad_host[b],
            "h": h_host[b],
            "c": c_host[b],
            **packed,
        })
    nc = _get_nc()
    kwargs = {}
    if trace:
        _enable_trace_hooks()
        kwargs = dict(trace=True, trace_cores=[0])
        if tmpdir:
            kwargs["tmpdir"] = tmpdir
    res = run_bass_kernel_spmd(nc, in_maps, core_ids=list(range(NCORES)), **kwargs)
    ch = np.stack([res.results[b]["och"].reshape(CH, HW, HW).astype(np.float32)
                   for b in range(B)])
    cc = np.stack([res.results[b]["occ"].reshape(CH, HW, HW).astype(np.float32)
                   for b in range(B)])
    return (ch, cc), res


def kernel(**inputs):
    (ch, cc), _ = run(inputs, trace=False)
    return ch, cc


# ---------- optional NTFF tracing support (test harness only) ----------

def _enable_trace_hooks():
    import types, ctypes, contextlib
    if "antenv.axon_hooks" in sys.modules:
        return
    import concourse.bass_utils as bass_utils

    def _ntff_profile_via_ctypes(so_path):
        lib = ctypes.CDLL(so_path)
        if not hasattr(lib, "axon_start_nrt_profile"):
            return None
        lib.axon_start_nrt_profile.argtypes = [
            ctypes.POINTER(ctypes.c_int64), ctypes.c_size_t]
        lib.axon_start_nrt_profile.restype = ctypes.c_int64
        lib.axon_stop_nrt_profile.argtypes = [ctypes.c_char_p]
        lib.axon_stop_nrt_profile.restype = ctypes.c_int64

        @contextlib.contextmanager
        def _hook(output_dir, device_ids):
            import jax
            jax.devices()
            if device_ids:
                ids = (ctypes.c_int64 * len(device_ids))(*device_ids)
                rc = lib.axon_start_nrt_profile(ids, len(device_ids))
            else:
                rc = lib.axon_start_nrt_profile(None, 0)
            if rc != 0:
                raise RuntimeError(f"axon_start_nrt_profile rc={rc}")
            try:
                yield
            finally:
                lib.axon_stop_nrt_profile(str(output_dir).encode())
        return _hook

    hook = _ntff_profile_via_ctypes("/opt/axon/libaxon_pjrt.so")
    mod = types.ModuleType("antenv.axon_hooks")
    mod.get_axon_ntff_profile_hook = lambda: hook
    mod.set_axon_ntff_profile_hook = lambda h: None
    sys.modules["antenv.axon_hooks"] = mod
    bass_utils.upload_artifacts = lambda tmpdir: "local://" + str(tmpdir)


# revision 40
# speedup vs baseline: 1.1875x; 1.1875x over previous
"""BottleneckLSTMCell fused kernel for 8 Trainium2 NeuronCores.

Sharding: data-parallel over batch (B=8 -> 1 image per core). Each core runs
the full cell for its image. ~273us HW (v2.12), from 364.7us baseline.

v2.12 design (every step trace-driven; see work/NOTES.md):
- Lag-2 pipeline: round r runs Wy for slab ra=r, dwi/gates/LSTM for slab
  rb=r-2, and the off-PE x-conv for slab rx=r+3 (slabs 0-2 precomputed as
  PE diag matmuls in the prologue while weights stream). The lag means the
  dwi diag MMs read i_pad rows written a full round earlier - PE never
  waits on same-round ACT writes.
- PE: Wy (28 MMs) + dwi diag (36; 27 from round 6 on) + gates (64, into
  2-bank PSUM pair tiles) + dwx chunk-0 diag (9, rounds 0-4) per round at
  ~216ns/MM, N=512. 22 dummy warm-up MMs at t=0 hold the HAM activity
  window so real MMs start at K=8/8 (2.4 GHz); it stays warm to the end.
- ACT: i_pad writes (Identity+bias from PSUM), b copies, sigmoids batched
  per bank-pair ((i,f) as one FD=1024 ACTIVATE), relu for the c gate.
  Keeping ACT light matters most: it is the PSUM-recycle path that paces
  the PE (overloading it in fill rounds was the main v2.4-v2.9 stall).
- DVE: dwx chunk 1/2 product+add chains (rounds 0-4); from round 6 the
  chains are done and dwi chunk 0 moves here (9 tensor_mul against
  broadcast tap weights + add chain) - its weights are DMA'd OVER dwxb
  slots 0-8 once the last chain has read them (extending this offload to
  round 4/5, where DVE still has chains, regressed badly - don't). LSTM
  pointwise all bf16 (u1/u2/cc/clip via 2-op tensor_scalar/ch).
- Outputs cc/ch are written bf16 (halves store DMA); host casts to f32.
  ch stores ride the ACT DMA queue, everything else the sync queue.
- x loads are single contiguous 1320B-per-partition lines (a slab's 10
  rows are contiguous in the flat 66x66 image); the strided [10,66] form
  generated 1280 tiny 132B descriptors and arrived ~3x late, which
  stalled the ACT queue head and cascaded into PE gaps.
"""

import sys

if '/opt/trn_rl_repo' not in sys.path:
    sys.path.insert(0, '/opt/trn_rl_repo')

import numpy as np
import ml_dtypes

import concourse.bass as bass  # noqa: F401
from concourse import bacc
import concourse.mybir as mybir
from concourse.tile import TileContext
from concourse.bass_utils import run_bass_kernel_spmd

F32 = mybir.dt.float32
BF = mybir.dt.bfloat16
AF = mybir.ActivationFunctionType
ALU = mybir.AluOpType
BF_NP = ml_dtypes.bfloat16

B, CIN, CH, HW = 8, 320, 512, 64
PIX = HW * HW          # 4096
NCORES = 8
NCHUNK = 8             # spatial slabs of 8 rows (512 px)
XCH = [128, 128, 64]   # x channel chunk sizes (320)

TAPS = [(t // 3 - 1, t % 3 - 1) for t in range(9)]
FLAT = "p a b -> p (a b)"


def build_nc():
    nc = bacc.Bacc(None, target_bir_lowering=False, num_devices=NCORES)

    xd = nc.dram_tensor("x", (CIN, 66 * 66), BF, kind="ExternalInput")
    hd = nc.dram_tensor("h", (128, 4, PIX), BF, kind="ExternalInput")
    cd = nc.dram_tensor("c", (128, 4, PIX), BF, kind="ExternalInput")
    wyd = nc.dram_tensor("wy", (128, 7, 512), BF, kind="ExternalInput")
    wybd = nc.dram_tensor("wyb", (128, 4), F32, kind="ExternalInput")
    wgd = nc.dram_tensor("wg", (128, 16, 512), BF, kind="ExternalInput")
    dwidd = nc.dram_tensor("dwid", (128, 4, 1152), BF, kind="ExternalInput")
    dwxdgd = nc.dram_tensor("dwxdg", (128, 3, 1152), BF, kind="ExternalInput")
    dwxbd = nc.dram_tensor("dwxb", (128, 18, 528), BF, kind="ExternalInput")
    dwibd = nc.dram_tensor("dwib", (128, 9, 528), BF, kind="ExternalInput")
    dwib2d = nc.dram_tensor("dwib2", (128, 9, 528), BF, kind="ExternalInput")
    ccd = nc.dram_tensor("occ", (CH, PIX), BF, kind="ExternalOutput")
    chd = nc.dram_tensor("och", (CH, PIX), BF, kind="ExternalOutput")

    x_ap, h_ap, c_ap = xd.ap(), hd.ap(), cd.ap()
    cc_ap, ch_ap = ccd.ap(), chd.ap()

    with TileContext(nc) as tc:
        with tc.tile_pool(name="persist", bufs=1) as pp, \
             tc.tile_pool(name="sA", bufs=2) as sA, \
             tc.tile_pool(name="sB", bufs=2) as sB, \
             tc.tile_pool(name="psi", bufs=2, space="PSUM") as psi_p, \
             tc.tile_pool(name="psg", bufs=2, space="PSUM") as psg_p, \
             tc.tile_pool(name="psb", bufs=2, space="PSUM") as psb_p:

            # ------------- persistent tiles -------------
            i_pad = [pp.tile([128, 66, 66], BF, tag=f"ipad{m}", name=f"ipad{m}")
                     for m in range(4)]
            wy_t = pp.tile([128, 7, 512], BF, tag="wy", name="wy")
            wyb_t = pp.tile([128, 4], F32, tag="wyb", name="wyb")
            wg_t = pp.tile([128, 16, 512], BF, tag="wg", name="wg")
            dwid_t = pp.tile([128, 4, 1152], BF, tag="dwid", name="dwid")
            dwxdg_t = pp.tile([128, 3, 1152], BF, tag="dwxdg", name="dwxdg")
            dwxb_t = pp.tile([128, 18, 528], BF, tag="dwxb", name="dwxb")
            dwib2_t = pp.tile([128, 9, 528], BF, tag="dwib2", name="dwib2")
            scr_t = pp.tile([128, 4], BF, tag="scr", name="scr")
            junk_t = pp.tile([128, 512], BF, tag="junk", name="junk")

            # ------------- helpers -------------
            def dma_h(n):
                t = sB.tile([128, 4, 512], BF, tag="h", name="h")
                nc.sync.dma_start(out=t[:], in_=h_ap[:, :, 512 * n:512 * (n + 1)])
                return t

            def dma_c(n):
                t = sB.tile([128, 4, 512], BF, tag="c", name="c")
                nc.sync.dma_start(out=t[:], in_=c_ap[:, :, 512 * n:512 * (n + 1)])
                return t

            def dma_xpad(n, eng=None):
                # a slab's 10 rows are contiguous in DRAM (flat 66*66 image),
                # so each load is one 1320B line per partition instead of
                # 10 tiny 132B descriptors
                eng = eng or nc.sync
                ts = []
                for ci in range(3):
                    pc = XCH[ci]
                    xp = sA.tile([128, 10, 66], BF, tag=f"xp{ci}",
                                 name=f"xp{ci}", bufs=2)
                    eng.dma_start(
                        out=xp[:pc, :, :].rearrange(FLAT),
                        in_=x_ap[128 * ci:128 * ci + pc,
                                 528 * n:528 * n + 660])
                    ts.append(xp)
                return ts

            def xwin(xp, pc, t):
                dy, dx = TAPS[t]
                return xp[:pc, 1 + dy:9 + dy, 1 + dx:65 + dx]

            def iwin(ci, n, t):
                dy, dx = TAPS[t]
                r0 = 8 * n
                return i_pad[ci][:, 1 + r0 + dy:9 + r0 + dy, 1 + dx:65 + dx]

            def tt_chain(tag, xw_tile, xp, ci, pc=128):
                """dwx chunk on DVE: 9 bf16 tensor_mul products against the
                pre-broadcast weight tiles, then an 8-add chain. All
                contiguous flat ops (elements 1..526)."""
                prod = sA.tile([128, 9, 528], BF, tag="pd",
                               name="pd", bufs=1)
                xf = xp[:pc, :, :].rearrange(FLAT)
                of = xw_tile[:pc, :, :].rearrange(FLAT)
                for t in range(9):
                    dy, dx = TAPS[t]
                    s = 66 * (1 + dy) + dx
                    nc.vector.tensor_mul(
                        prod[:pc, t, 0:526], xf[:, s + 1:s + 527],
                        dwxb_t[:pc, 9 * (ci - 1) + t, 0:526])
                tmps = [sA.tile([128, 528], BF, tag=f"tc{tag}{j}",
                                name=f"tc{tag}{j}") for j in range(2)]
                nc.vector.tensor_add(tmps[0][:pc, 0:526], prod[:pc, 0, 0:526],
                                     prod[:pc, 1, 0:526])
                for j in range(2, 8):
                    nc.vector.tensor_add(tmps[(j + 1) % 2][:pc, 0:526],
                                         tmps[j % 2][:pc, 0:526],
                                         prod[:pc, j, 0:526])
                nc.vector.tensor_add(of[:, 0:526], tmps[0][:pc, 0:526],
                                     prod[:pc, 8, 0:526])

            def new_xw():
                # 4 generations alive: slab ra (reading), ra+1/ra+2 (ready),
                # rx=ra+3 (being written)
                return [sA.tile([128, 8, 66], BF, tag=f"xw{ci}",
                                name=f"xw{ci}", bufs=3) for ci in range(3)]

            def b_dve(n, ci, wtile):
                """dwi chunk ci of slab n on DVE: 9 tensor_mul products
                against broadcast tap weights + add chain into a grid-layout
                tile (interior at cols 0:64)."""
                pr = sA.tile([128, 9, 528], BF, tag="pd", name="pd", bufs=1)
                r0 = 8 * n
                ipf = i_pad[ci][:, r0:r0 + 10, :].rearrange(FLAT)
                for t in range(9):
                    dy, dx = TAPS[t]
                    s = 66 * (1 + dy) + dx
                    nc.vector.tensor_mul(
                        pr[:, t, 0:526], ipf[:, s + 1:s + 527],
                        wtile[:, t, 0:526])
                bg = sB.tile([128, 8, 66], BF, tag=f"bg{ci}", name=f"bg{ci}")
                btm = [sA.tile([128, 528], BF, tag=f"b0t{j}",
                               name=f"b0t{j}", bufs=1) for j in range(2)]
                nc.vector.tensor_add(btm[0][:, 0:526], pr[:, 0, 0:526],
                                     pr[:, 1, 0:526])
                for j in range(2, 8):
                    nc.vector.tensor_add(btm[(j + 1) % 2][:, 0:526],
                                         btm[j % 2][:, 0:526],
                                         pr[:, j, 0:526])
                nc.vector.tensor_add(
                    bg[:, :, :].rearrange(FLAT)[:, 0:526],
                    btm[0][:, 0:526], pr[:, 8, 0:526])
                return bg

            def x0_emit(n, xp0, xw0):
                """dwx chunk 0 of slab n as PE diag matmuls + one ACT copy
                (9 MMs ~= 1.9us PE; the 9-ACT-product + 8-Pool-add version
                overloaded ACT in exactly the pipeline-fill rounds)."""
                psb = psb_p.tile([128, 8, 64], F32, tag="psb", name="psb")
                for t in range(9):
                    nc.tensor.matmul(
                        psb[:, :, :], dwxdg_t[:, 0, 128 * t:128 * (t + 1)],
                        xwin(xp0, 128, t), start=(t == 0), stop=(t == 8))
                nc.scalar.copy(xw0[:, :, 0:64], psb[:, :, :])

            def emit_wy(n, h_sb, xw_sb):
                """i = Wy @ [h; xw] + bias -> i_pad interior rows (bf16)."""
                r0 = 8 * n
                for m in range(4):
                    ps = psi_p.tile([128, 512], F32, tag="psi", name="psi")
                    for k in range(4):
                        nc.tensor.matmul(
                            ps[:, :], wy_t[:, k, 128 * m:128 * (m + 1)],
                            h_sb[:, k, :], start=(k == 0), stop=False)
                    for j in range(3):
                        pc = XCH[j]
                        nc.tensor.matmul(
                            ps[:, :], wy_t[:pc, 4 + j, 128 * m:128 * (m + 1)],
                            xw_sb[j][:pc, :, 0:64], start=False, stop=(j == 2))
                    nc.scalar.activation(
                        i_pad[m][:, 1 + r0:9 + r0, 1:65], ps[:, :],
                        AF.Identity, bias=wyb_t[:, m:m + 1], scale=1.0)

            # ---------------- prologue ----------------
            # PE warm-up: ~20 dummy matmuls on a junk tile keep the PE HAM
            # activity window busy while the first DMAs stream, so the real
            # matmuls start at K=8/8 (2.4 GHz) instead of cold.
            nc.vector.memset(junk_t[:, :], 0.0)
            for _ in range(22):
                psw = psb_p.tile([128, 8, 64], F32, tag="psb", name="psb")
                nc.tensor.matmul(psw[:, :, :], junk_t[:, 0:128],
                                 junk_t[:, :].rearrange("p (a b) -> p a b",
                                                        a=8, b=64),
                                 start=True, stop=True)

            # DMA order: first the tensors gating the first diag MMs, then
            # the Wy weights, then everything else. x loads go through the
            # ACT DMA queue in parallel with the sync-queue weights.
            # interleave diag-weight and x chunks so the first diag MM's two
            # inputs are the first two transfers on the queue
            xp0 = []
            for ci in range(3):
                nc.sync.dma_start(out=dwxdg_t[:, ci, :],
                                  in_=dwxdgd.ap()[:, ci, :])
                pc = XCH[ci]
                xp = sA.tile([128, 10, 66], BF, tag=f"xp{ci}",
                             name=f"xp{ci}", bufs=2)
                nc.sync.dma_start(out=xp[:pc, :, :].rearrange(FLAT),
                                  in_=x_ap[128 * ci:128 * ci + pc, 0:660])
                xp0.append(xp)
            xp1 = dma_xpad(1)
            nc.sync.dma_start(out=wy_t[:], in_=wyd.ap())
            h0 = dma_h(0)
            nc.sync.dma_start(out=wyb_t[:], in_=wybd.ap())
            xp2 = dma_xpad(2)
            nc.sync.dma_start(out=dwid_t[:], in_=dwidd.ap())
            nc.sync.dma_start(out=wg_t[:], in_=wgd.ap())
            nc.sync.dma_start(out=dwxb_t[:], in_=dwxbd.ap())
            nc.sync.dma_start(out=dwib2_t[:], in_=dwib2d.ap())
            xp3 = dma_xpad(3)

            # warm the ACT function tables while DMAs stream
            nc.vector.memset(scr_t[:, :], 0.0)
            nc.scalar.activation(scr_t[:, 0:1], scr_t[:, 1:2], AF.Sigmoid)
            nc.scalar.activation(scr_t[:, 2:3], scr_t[:, 3:4], AF.Relu)

            # zero i_pad halo borders (rows 0/65, cols 0/65)
            for m in range(4):
                eng = nc.vector if m < 2 else nc.gpsimd
                eng.memset(i_pad[m][:, 0, :], 0.0)
                eng.memset(i_pad[m][:, 65, :], 0.0)
                eng.memset(i_pad[m][:, 1:65, 0], 0.0)
                eng.memset(i_pad[m][:, 1:65, 65], 0.0)

            # prologue x-conv for slabs 0-2 entirely as PE diag matmuls
            # (PE is otherwise idle while Wy/gate weights stream in); ACT
            # copies PSUM -> xw bf16
            def dwx_diag(xps):
                xw = new_xw()
                for ci in range(3):
                    pc = XCH[ci]
                    psb = psb_p.tile([128, 8, 64], F32, tag="psb", name="psb")
                    for t in range(9):
                        nc.tensor.matmul(
                            psb[:pc, :, :],
                            dwxdg_t[:pc, ci, pc * t:pc * (t + 1)],
                            xwin(xps[ci], pc, t), start=(t == 0), stop=(t == 8))
                    nc.scalar.copy(xw[ci][:pc, :, 0:64], psb[:pc, :, :])
                return xw

            xw_sb = {0: dwx_diag(xp0), 1: dwx_diag(xp1), 2: dwx_diag(xp2)}
            h_sb = {0: h0}
            xp_sb = {3: xp3}
            b1_pre = {}

            # ---------------- rounds ----------------
            # ra = Wy stage slab, rb = dwi/gate/LSTM stage slab (lag 2),
            # rx = dwx stage slab (3 ahead; slabs 0-2 precomputed)
            for r in range(NCHUNK + 2):
                ra = r
                rb = r - 2
                rx = r + 3

                # DMAs for this round
                if rb >= 0:
                    c_t = dma_c(rb)
                if ra + 1 < NCHUNK:
                    h_sb[ra + 1] = dma_h(ra + 1)
                if rx + 1 < NCHUNK:
                    xp_sb[rx + 1] = dma_xpad(rx + 1)

                # PE: Wy matmuls + i_pad for slab ra.
                if ra < NCHUNK:
                    emit_wy(ra, h_sb.pop(ra), xw_sb.pop(ra))

                # From round 6 on the DVE chains are finished, so dwi chunk 0
                # moves to the idle DVE (9 tensor_mul products + add chain),
                # shaving 9 diag MMs off the PE. The i-tap broadcast weights
                # are DMA'd over dwxb slots 0-8 (free after the last chain)
                # at round 5.
                offload_b0 = rb >= 3
                if rb >= 0 and offload_b0:
                    b0 = b_dve(rb, 0, dwxb_t)
                # dwi chunk 1 of the NEXT gate slab, one round early: its
                # i_pad rows complete with this round's ipad m1, so it gets
                # a full round of slack on the DVE (only in chain-free
                # rounds, and not in the short tail rounds)
                nb = r - 1
                if 4 <= nb <= 6:
                    b1_pre[nb] = b_dve(nb, 1, dwib2_t)

                # DVE: dwx chunks 1/2 of slab rx
                if rx < NCHUNK:
                    xw_next = new_xw()
                    tt_chain("xc1", xw_next[1], xp_sb[rx][1], 1)
                    tt_chain("xc2", xw_next[2], xp_sb[rx][2], 2, pc=64)
                if r == 4:
                    # overwrite dwxb slots 0-8 with the i-chunk0 tap
                    # broadcasts once the last dwx chain has read them
                    nc.sync.dma_start(out=dwxb_t[:, 0:9, :],
                                      in_=dwibd.ap())

                # PE: dwi chunks of slab rb as diag matmuls; i_pad rows for
                # rb+1 (the bottom halo) were written LAST round, so no
                # same-round ACT dependency.
                if rb >= 0:
                    offload_b1 = 4 <= rb <= 6
                    bt = {}
                    for ci in range(4):
                        if (offload_b0 and ci == 0) or \
                           (offload_b1 and ci == 1):
                            continue
                        psb = psb_p.tile([128, 8, 64], F32, tag="psb",
                                         name="psb")
                        for t in range(9):
                            nc.tensor.matmul(
                                psb[:, :, :],
                                dwid_t[:, ci, 128 * t:128 * (t + 1)],
                                iwin(ci, rb, t), start=(t == 0), stop=(t == 8))
                        b = sB.tile([128, 8, 64], BF, tag=f"b{ci}",
                                    name=f"b{ci}")
                        nc.scalar.copy(b[:, :, :], psb[:, :, :])
                        bt[ci] = b[:, :, :]
                    if offload_b0:
                        bt[0] = b0[:, :, 0:64]
                    if offload_b1:
                        bt[1] = b1_pre.pop(rb)[:, :, 0:64]

                    # PE gates (paired PSUM banks) + ACT batched sigmoids +
                    # DVE LSTM pointwise per m-chunk. When b0 is off-PE the
                    # contraction order ends with chunk 0 for maximum slack.
                    KORD = (1, 2, 3, 0) if offload_b0 else (0, 1, 2, 3)
                    for m in range(4):
                        # pair A: gates (i, f) -> one batched sigmoid
                        pgA = psg_p.tile([128, 2, 512], F32, tag="psg",
                                         name="psg")
                        for g in range(2):
                            for j, k in enumerate(KORD):
                                nc.tensor.matmul(
                                    pgA[:, g, :],
                                    wg_t[:, 4 * g + k, 128 * m:128 * (m + 1)],
                                    bt[k],
                                    start=(j == 0), stop=(j == 3))
                        st_if = sB.tile([128, 2, 512], BF, tag="sif",
                                        name="sif", bufs=3)
                        nc.scalar.activation(st_if[:, :, :], pgA[:, :, :],
                                             AF.Sigmoid)
                        # pair B: gates (o, c) -> sigmoid + relu
                        pgB = psg_p.tile([128, 2, 512], F32, tag="psg",
                                         name="psg")
                        for g in range(2, 4):
                            for j, k in enumerate(KORD):
                                nc.tensor.matmul(
                                    pgB[:, g - 2, :],
                                    wg_t[:, 4 * g + k, 128 * m:128 * (m + 1)],
                                    bt[k],
                                    start=(j == 0), stop=(j == 3))
                        st_o = sB.tile([128, 512], BF, tag="so",
                                       name="so", bufs=3)
                        nc.scalar.activation(st_o[:, :], pgB[:, 0, :],
                                             AF.Sigmoid)
                        st_cr = sB.tile([128, 512], BF, tag="scr2",
                                        name="scr2", bufs=3)
                        nc.scalar.activation(st_cr[:, :], pgB[:, 1, :],
                                             AF.Relu)

                        # DVE LSTM: all bf16; none of it holds PSUM banks
                        u1 = sB.tile([128, 512], BF, tag="u1", name="u1",
                                     bufs=3)
                        nc.vector.tensor_mul(u1[:, :], st_if[:, 1, :],
                                             c_t[:, m, :])
                        u2 = sB.tile([128, 512], BF, tag="u2", name="u2",
                                     bufs=3)
                        nc.vector.scalar_tensor_tensor(
                            out=u2[:, :], in0=st_cr[:, :], scalar=6.0,
                            in1=st_if[:, 0, :], op0=ALU.min, op1=ALU.mult)
                        cc_t = sB.tile([128, 512], BF, tag="cc",
                                       name="cc", bufs=3)
                        nc.vector.tensor_add(cc_t[:, :], u1[:, :], u2[:, :])
                        nc.sync.dma_start(
                            out=cc_ap[128 * m:128 * (m + 1),
                                      512 * rb:512 * (rb + 1)],
                            in_=cc_t[:])
                        rcc = sB.tile([128, 512], BF, tag="rcc",
                                      name="rcc", bufs=3)
                        nc.vector.tensor_scalar(
                            out=rcc[:, :], in0=cc_t[:, :], scalar1=6.0,
                            scalar2=0.0, op0=ALU.min, op1=ALU.max)
                        ch_t = sB.tile([128, 512], BF, tag="ch",
                                       name="ch", bufs=3)
                        nc.vector.tensor_mul(ch_t[:, :], rcc[:, :],
                                             st_o[:, :])
                        nc.scalar.dma_start(
                            out=ch_ap[128 * m:128 * (m + 1),
                                      512 * rb:512 * (rb + 1)],
                            in_=ch_t[:])

                # dwx chunk 0 of slab rx: ACT products + Pool add chain,
                # deprioritized so the scheduler parks them behind the
                # latency-critical ACT/Pool work (3 rounds of slack).
                if rx < NCHUNK:
                    with tc.high_priority(offset=-4000):
                        x0_emit(rx, xp_sb[rx][0], xw_next[0])
                    xw_sb[rx] = xw_next
                    del xp_sb[rx]

    nc.compile()
    return nc


def pack_weights(W_dw, W_dwb, Wy, Wy_b, Wi, Wbi, Wbf, Wbc, Wbo):
    WyT = Wy[:, :, 0, 0].T.astype(np.float32)  # (832, 512) lhsT
    wy = np.zeros((128, 7, 512), np.float32)
    for k in range(4):  # h chunks first
        wy[:, k, :] = WyT[320 + 128 * k:320 + 128 * (k + 1), :]
    for k in range(2):
        wy[:, 4 + k, :] = WyT[128 * k:128 * (k + 1), :]
    wy[:64, 6, :] = WyT[256:320, :]

    wyb = (Wy_b + Wy[:, :320, 0, 0] @ W_dwb).astype(np.float32)
    wyb = np.ascontiguousarray(wyb.reshape(4, 128).T)

    # gate order (i, f, o, c) so (i,f) and (o,c) share PSUM bank pairs
    wg = np.zeros((128, 16, 512), np.float32)
    for g, W in enumerate([Wbi, Wbf, Wbo, Wbc]):
        lhsT = W[:, :, 0, 0].T.astype(np.float32)  # (512 in, 512 out)
        for k in range(4):
            wg[:, 4 * g + k, :] = lhsT[128 * k:128 * (k + 1), :]

    wtap_x = W_dw[:, 0].reshape(CIN, 9)  # (c, t) tap-major (dy,dx)

    # pre-broadcast bf16 tap weights for the DVE tensor_mul products:
    # slots 0-17 = x chunks 1/2, slots 18-26 = i chunk 0
    wtap_i = Wi[:, 0].reshape(CH, 9)
    dwxb = np.zeros((128, 18, 528), np.float32)
    for ci in (1, 2):
        pc = XCH[ci]
        for t in range(9):
            dwxb[:pc, 9 * (ci - 1) + t, :] = \
                wtap_x[128 * ci:128 * ci + pc, t][:, None]

    dwib = np.zeros((128, 9, 528), np.float32)
    dwib2 = np.zeros((128, 9, 528), np.float32)
    for t in range(9):
        dwib[:, t, :] = wtap_i[:128, t][:, None]
        dwib2[:, t, :] = wtap_i[128:256, t][:, None]

    dwid = np.zeros((128, 4, 1152), np.float32)
    idx = np.arange(128)
    for ci in range(4):
        for t in range(9):
            dwid[idx, ci, 128 * t + idx] = wtap_i[128 * ci + idx, t]
    dwxdg = np.zeros((128, 3, 1152), np.float32)
    for ci in range(3):
        pc = XCH[ci]
        ic = np.arange(pc)
        for t in range(9):
            dwxdg[ic, ci, pc * t + ic] = wtap_x[128 * ci + ic, t]

    return {
        "wy": wy.astype(BF_NP), "wyb": wyb, "wg": wg.astype(BF_NP),
        "dwid": dwid.astype(BF_NP),
        "dwxdg": dwxdg.astype(BF_NP), "dwxb": dwxb.astype(BF_NP),
        "dwib": dwib.astype(BF_NP), "dwib2": dwib2.astype(BF_NP),
    }


_CACHE = {}


def _get_nc():
    if "nc" not in _CACHE:
        _CACHE["nc"] = build_nc()
    return _CACHE["nc"]


def run(inputs, trace=False, tmpdir=None):
    """inputs: dict as from setup_inputs(). Returns ((ch, cc), results_obj)."""
    inp = {k: np.asarray(v, np.float32) for k, v in inputs.items()}
    packed = pack_weights(
        inp["W_dw"], inp["W_dwb"], inp["Wy"], inp["Wy_b"], inp["Wi"],
        inp["Wbi"], inp["Wbf"], inp["Wbc"], inp["Wbo"],
    )
    xpad_host = np.zeros((B, CIN, 66, 66), np.float32)
    xpad_host[:, :, 1:65, 1:65] = inp["x"]
    xpad_host = xpad_host.astype(BF_NP).reshape(B, CIN, 66 * 66)
    # h/c device layout: [c_low(128), k_chunk(4), pix]
    h_host = np.ascontiguousarray(
        inp["h"].reshape(B, 4, 128, PIX).transpose(0, 2, 1, 3)).astype(BF_NP)
    c_host = np.ascontiguousarray(
        inp["c"].reshape(B, 4, 128, PIX).transpose(0, 2, 1, 3)).astype(BF_NP)
    in_maps = []
    for b in range(B):
        in_maps.append({
            "x": xpad_host[b],
            "h": h_host[b],
            "c": c_host[b],
            **packed,
        })
    nc = _get_nc()
    kwargs = {}
    if trace:
        _enable_trace_hooks()
        kwargs = dict(trace=True, trace_cores=[0])
        if tmpdir:
            kwargs["tmpdir"] = tmpdir
    res = run_bass_kernel_spmd(nc, in_maps, core_ids=list(range(NCORES)), **kwargs)
    ch = np.stack([res.results[b]["och"].reshape(CH, HW, HW).astype(np.float32)
                   for b in range(B)])
    cc = np.stack([res.results[b]["occ"].reshape(CH, HW, HW).astype(np.float32)
                   for b in range(B)])
    return (ch, cc), res


def kernel(**inputs):
    (ch, cc), _ = run(inputs, trace=False)
    return ch, cc


# ---------- optional NTFF tracing support (test harness only) ----------

def _enable_trace_hooks():
    import types, ctypes, contextlib
    if "antenv.axon_hooks" in sys.modules:
        return
    import concourse.bass_utils as bass_utils

    def _ntff_profile_via_ctypes(so_path):
        lib = ctypes.CDLL(so_path)
        if not hasattr(lib, "axon_start_nrt_profile"):
            return None
        lib.axon_start_nrt_profile.argtypes = [
            ctypes.POINTER(ctypes.c_int64), ctypes.c_size_t]
        lib.axon_start_nrt_profile.restype = ctypes.c_int64
        lib.axon_stop_nrt_profile.argtypes = [ctypes.c_char_p]
        lib.axon_stop_nrt_profile.restype = ctypes.c_int64

        @contextlib.contextmanager
        def _hook(output_dir, device_ids):
            import jax
            jax.devices()
            if device_ids:
                ids = (ctypes.c_int64 * len(device_ids))(*device_ids)
                rc = lib.axon_start_nrt_profile(ids, len(device_ids))
            else:
                rc = lib.axon_start_nrt_profile(None, 0)
            if rc != 0:
                raise RuntimeError(f"axon_start_nrt_profile rc={rc}")
            try:
                yield
            finally:
                lib.axon_stop_nrt_profile(str(output_dir).encode())
        return _hook

    hook = _ntff_profile_via_ctypes("/opt/axon/libaxon_pjrt.so")
    mod = types.ModuleType("antenv.axon_hooks")
    mod.get_axon_ntff_profile_hook = lambda: hook
    mod.set_axon_ntff_profile_hook = lambda h: None
    sys.modules["antenv.axon_hooks"] = mod
    bass_utils.upload_artifacts = lambda tmpdir: "local://" + str(tmpdir)
